# revision 21
# baseline (speedup 1.0000x reference)
"""GNN message-passing (std aggregator) on 8 TRN2 NeuronCores.

Math per target node: count, S1 = sum x[src], S2 = sum x[src]^2;
mean = S1/max(count,eps); var = S2/count - mean^2;
std = sqrt(max(var,0)), zeroed where count <= 1.

Strategy: shard TARGET nodes across cores (no collectives). Host packs nodes
into 128-bin blocks with a greedy 4-dim balancer (per-quarter loads <= ~512),
sorts blocks by load and deals them serpentine to cores so every core has the
same per-position load profile. Each block position gets its own compile-time
capacity (128-multiple), so gather padding is ~2-3% instead of 25%. Per group
of GB blocks and src-quarter q there is ONE dma_gather (int16 idx < 25000);
gathers round-robin 4 SWDGE queues so 4 GpSimd Q7 pairs emit descriptors
concurrently (~3.2x). Per group: ACT builds [x | x^2 | 1] bf16 rhs, DVE builds
one-hot tiles (label-vs-iota is_equal), PE accumulates [128 x 129] = [S1 | S2
| count] per block in PSUM, then a batched finishing pass computes std and one
strided DMA per group writes out.
"""

import numpy as np

N_NODES = 100000
N_FEAT = 64
N_EDGES = 1600000
P = 128
NCORES = 8
NB = 98                 # blocks per core
NBLK = NCORES * NB      # 784
GB = 7                  # blocks per group; 98 = 14*7
NG = NB // GB
NQUART = 4
NQ = N_NODES // NQUART  # rows per src quarter (25000 < 32768 for int16 idx)
EPS = 1e-8
MM_DT = "bfloat16"      # matmul operand dtype

_CACHE = {}


def _build_program(caps, mm_dt):
    """caps: tuple of NB ints, capacity (multiple of 128) per block position."""
    import concourse.bacc as bacc
    import concourse.mybir as mybir
    import concourse.tile as tile

    F32 = mybir.dt.float32
    I16 = mybir.dt.int16
    MDT = getattr(mybir.dt, mm_dt)
    AO = mybir.AluOpType
    AF = mybir.ActivationFunctionType

    f = N_FEAT
    W = 2 * f + 1
    tiles = [c // P for c in caps]               # tile-columns per (pos, q)
    # per-group geometry
    gtiles = [sum(tiles[g * GB:(g + 1) * GB]) for g in range(NG)]  # per q
    gcols_g = [4 * t for t in gtiles]            # tile-cols per group
    maxgt = max(gtiles)
    maxgc = max(gcols_g)
    C = sum(gcols_g)                             # total columns per core
    i16_gq = [t * P // 16 for t in gtiles]       # idx16 cols per (g, q) gather
    IC = 4 * sum(i16_gq)                         # idx16 cols per core

    nc = bacc.Bacc(num_swdge_queues=4)
    xd = nc.declare_dram_parameter("x", [N_NODES, f], F32, isOutput=False)
    gidxd = nc.declare_dram_parameter("gidx", [P, IC], I16, isOutput=False)
    tgtd = nc.declare_dram_parameter("tgt", [P, C], MDT, isOutput=False)
    outd = nc.declare_dram_parameter("out", [NB * P, f], F32, isOutput=True)

    with tile.TileContext(nc) as tc:
        with (
            tc.tile_pool(name="const", bufs=1) as constp,
            tc.tile_pool(name="msg", bufs=2) as msgp,
            tc.tile_pool(name="oh", bufs=2) as ohp,
            tc.tile_pool(name="fin", bufs=2) as finp,
            tc.tile_pool(name="ov", bufs=2) as ovp,
            tc.tile_pool(name="ps", bufs=8, space="PSUM") as psump,
        ):
            iotat = constp.tile([P, maxgt * P], MDT)
            nc.gpsimd.iota(iotat[:], pattern=[[0, maxgt], [1, P]], base=0,
                           channel_multiplier=0,
                           allow_small_or_imprecise_dtypes=True)

            idxall = constp.tile([P, IC], I16)
            c0 = 4 * i16_gq[0]
            nc.sync.dma_start(out=idxall[:, 0:c0], in_=gidxd[:, 0:c0])
            nc.sync.dma_start(out=idxall[:, c0:], in_=gidxd[:, c0:])
            tgall = constp.tile([P, C], MDT)
            nc.sync.dma_start(out=tgall[:], in_=tgtd[:, :])

            out3 = outd[:].rearrange("(b p) f -> p b f", p=P)

            def _drain(pst):
                fin = finp.tile([P, GB * W], F32, tag="fin")
                for j, pt in enumerate(pst):
                    nc.scalar.activation(out=fin[:, j * W:(j + 1) * W],
                                         in_=pt[:], func=AF.Copy)
                return fin

            def _math(fin, ooff):
                f3 = fin[:].rearrange("p (b w) -> p b w", w=W)
                rec = finp.tile([P, GB], F32, tag="rec")
                nc.vector.tensor_scalar(
                    out=rec[:].rearrange("p (b u) -> p b u", u=1),
                    in0=f3[:, :, 2 * f:2 * f + 1],
                    scalar1=float(EPS), scalar2=None, op0=AO.add)
                nc.vector.reciprocal(out=rec[:], in_=rec[:])
                r3 = rec[:].rearrange("p (b u) -> p b u", u=1)
                mom = finp.tile([P, GB * 2 * f], F32, tag="mom")
                m3 = mom[:].rearrange("p (b w) -> p b w", w=2 * f)
                nc.vector.tensor_tensor(
                    out=m3[:, :, :], in0=f3[:, :, 0:2 * f],
                    in1=r3.to_broadcast([P, GB, 2 * f]), op=AO.mult)
                var = finp.tile([P, GB * f], F32, tag="var")
                v3 = var[:].rearrange("p (b w) -> p b w", w=f)
                nc.vector.tensor_tensor(
                    out=v3[:, :, :], in0=m3[:, :, 0:f], in1=m3[:, :, 0:f],
                    op=AO.mult)
                nc.vector.tensor_tensor(
                    out=v3[:, :, :], in0=m3[:, :, f:2 * f], in1=v3[:, :, :],
                    op=AO.subtract)
                std = ovp.tile([P, GB * f], F32, tag="std")
                nc.scalar.activation(out=std[:], in_=var[:], func=AF.Relu)
                nc.scalar.sqrt(out=std[:], in_=std[:])
                mask = finp.tile([P, GB], F32, tag="mask")
                nc.vector.tensor_scalar(
                    out=mask[:].rearrange("p (b u) -> p b u", u=1),
                    in0=f3[:, :, 2 * f:2 * f + 1],
                    scalar1=1.5, scalar2=None, op0=AO.is_gt)
                s3o = std[:].rearrange("p (b w) -> p b w", w=f)
                nc.vector.tensor_tensor(
                    out=s3o[:, :, :], in0=s3o[:, :, :],
                    in1=mask[:].rearrange("p (b u) -> p b u", u=1)
                        .to_broadcast([P, GB, f]),
                    op=AO.mult)
                nc.sync.dma_start(
                    out=out3[:, ooff:ooff + GB, :], in_=s3o[:, :, :])

            pending = []
            ioff = 0   # idx16 column offset
            coff = 0   # tgt column offset
            ooff = 0   # out block offset
            for g in range(NG):
                gt = gtiles[g]
                gc = gcols_g[g]
                i16g = i16_gq[g]

                if len(pending) == 2:
                    _math(*pending.pop(0))

                gbuf = msgp.tile([P, maxgc * f], F32, tag="g")
                g3 = gbuf[:].rearrange("p (c e) -> p c e", e=f)
                for q in range(NQUART):
                    nc.gpsimd.dma_gather(
                        out_ap=g3[:, q * gt:(q + 1) * gt, :],
                        in_ap=xd[q * NQ:(q + 1) * NQ, :],
                        idxs_ap=idxall[:, ioff + q * i16g:ioff + (q + 1) * i16g],
                        num_idxs=gt * P,
                        num_idxs_reg=gt * P,
                        elem_size=f,
                        single_packet=False,
                        queue_num=q,
                    )

                sqx = msgp.tile([P, maxgc * W], MDT, tag="sqx")
                s3 = sqx[:].rearrange("p (c w) -> p c w", w=W)
                pst = [psump.tile([P, W], F32, tag="ps",
                                  name=f"ps_{g}_{j}") for j in range(GB)]
                pss = [pt[:] for pt in pst]
                for q in range(NQUART):
                    sl = slice(q * gt, (q + 1) * gt)
                    nc.scalar.activation(out=s3[:, sl, 0:f], in_=g3[:, sl, :],
                                         func=AF.Copy)
                    nc.scalar.square(out=s3[:, sl, f:2 * f], in_=g3[:, sl, :])
                    nc.scalar.activation(out=s3[:, sl, 2 * f:W],
                                         in_=g3[:, sl, 0:1],
                                         func=AF.Copy, bias=1.0, scale=0.0)
                    oh = ohp.tile([P, maxgt * P], MDT)
                    nc.vector.tensor_tensor(
                        out=oh[:, 0:gt * P].rearrange("p (c e) -> p c e", e=P),
                        in0=tgall[:, coff + q * gt:coff + (q + 1) * gt]
                            .rearrange("p (c u) -> p c u", u=1)
                            .to_broadcast([P, gt, P]),
                        in1=iotat[:, 0:gt * P]
                            .rearrange("p (c e) -> p c e", e=P),
                        op=AO.is_equal,
                    )
                    toff = 0
                    for bl in range(GB):
                        nt = tiles[g * GB + bl]
                        for t in range(nt):
                            cl = q * gt + toff + t
                            nc.tensor.matmul(
                                out=pss[bl],
                                lhsT=oh[:, (toff + t) * P:(toff + t + 1) * P],
                                rhs=sqx[:, cl * W:(cl + 1) * W],
                                start=(q == 0 and t == 0),
                                stop=(q == NQUART - 1 and t == nt - 1),
                            )
                        toff += nt

                # finishing deferred one group so its DVE/ACT ops never
                # stall the next group's one-hot builds
                pending.append((_drain(pst), ooff))

                ioff += 4 * i16g
                coff += gc
                ooff += GB
            while pending:
                _math(*pending.pop(0))
    return nc


def _balance(deg4):
    """Greedy 4-dim balanced assignment of nodes to NBLK blocks (<=128 each)."""
    tot = deg4.sum(1)
    order = np.argsort(-tot, kind="stable")
    loads = np.zeros((NBLK, NQUART), np.int32)
    cnt = np.zeros(NBLK, np.int32)
    blk = np.empty(N_NODES, np.int64)
    slot = np.empty(N_NODES, np.int64)
    full = np.zeros(NBLK, bool)
    CAP = 512
    for n in order:
        cand = loads + deg4[n]
        mx = cand.max(axis=1)
        sc = np.where((cand > CAP).any(axis=1) | full, np.inf, mx)
        b = int(np.argmin(sc))
        if np.isinf(sc[b]):
            sc2 = np.where(full, np.inf, mx)
            b = int(np.argmin(sc2))
        blk[n] = b
        slot[n] = cnt[b]
        loads[b] += deg4[n]
        cnt[b] += 1
        if cnt[b] >= P:
            full[b] = True
    return blk, slot, loads


def _host_prep(x, edge_index):
    src = np.asarray(edge_index[0], dtype=np.int64)
    tgt = np.asarray(edge_index[1], dtype=np.int64)
    n_edges = src.shape[0]

    eq = src // NQ
    deg4 = np.bincount(tgt * NQUART + eq,
                       minlength=N_NODES * NQUART).reshape(N_NODES, NQUART)
    blk, slot, loads = _balance(deg4.astype(np.int32))

    # sort blocks by max quarter load desc, serpentine-deal to cores so each
    # core's position profile matches; capacity per position = max over cores
    bmax = loads.max(axis=1)
    border = np.argsort(-bmax, kind="stable")    # global block rank
    rank_of = np.empty(NBLK, np.int64)
    rank_of[border] = np.arange(NBLK)
    rounds = rank_of // NCORES
    posn = rank_of % NCORES
    core_of = np.where(rounds % 2 == 0, posn, NCORES - 1 - posn)
    pos_of = rounds                              # block position within core

    # per-position capacity (multiple of 128), same for all cores
    segmax = np.zeros(NB, np.int64)
    np.maximum.at(segmax, pos_of, bmax)
    caps = (np.ceil(np.maximum(segmax, 1) / P).astype(np.int64) * P)

    # per-edge placement
    eb = blk[tgt]
    ecore = core_of[eb]
    epos = pos_of[eb]
    es = slot[tgt]
    # segment id in stream order: (core, group, q, block-in-group)
    egrp = epos // GB
    ebl = epos % GB
    seg = ((ecore * NG + egrp) * NQUART + eq) * GB + ebl
    nseg = NCORES * NG * NQUART * GB
    # capacity per segment id
    segcap = np.empty(nseg, np.int64)
    sid = np.arange(nseg)
    segcap[:] = caps[(sid // (NQUART * GB)) % NG * GB + sid % GB]
    segstart = np.zeros(nseg, np.int64)
    np.cumsum(segcap[:-1], out=segstart[1:])
    total = int(segcap.sum())

    segsums = np.bincount(seg, minlength=nseg)
    assert (segsums <= segcap).all()

    order_e = np.argsort(seg, kind="stable")
    segs = seg[order_e]
    starts = np.zeros(nseg, np.int64)
    np.cumsum(segsums[:-1], out=starts[1:])
    within = np.arange(n_edges) - starts[segs]
    flat = segstart[segs] + within

    gidx_all = np.zeros(total, np.int16)
    tgt_f = np.full(total, -1.0, np.float32)
    gidx_all[flat] = (src[order_e] % NQ).astype(np.int16)
    tgt_f[flat] = es[order_e].astype(np.float32)
    # bf16 encoding: upper 16 bits of float32 (values 0..127/-1 are exact)
    tgt_all = (tgt_f.view(np.uint32) >> 16).astype(np.uint16).view(np.int16)

    xf = np.ascontiguousarray(np.asarray(x, dtype=np.float32))
    per_core = total // NCORES
    # per (core, g, q): gather stream = GB consecutive segments
    # idx16 wrap: [cap16, 16] -> [16, cap16], replicate x8
    tiles = caps // P
    gtiles = [int(tiles[g * GB:(g + 1) * GB].sum()) for g in range(NG)]

    in_maps = []
    for c in range(NCORES):
        base = c * per_core
        gi_parts = []
        tg_parts = []
        off = base
        for g in range(NG):
            for q in range(NQUART):
                L = gtiles[g] * P
                sidx = gidx_all[off:off + L]
                stgt = tgt_all[off:off + L]
                gi_parts.append(sidx.reshape(L // 16, 16).T)
                tg_parts.append(stgt.reshape(gtiles[g], P).T)
                off += L
        gi16 = np.concatenate(gi_parts, axis=1)          # [16, IC]
        idx16 = np.ascontiguousarray(np.tile(gi16, (8, 1)))
        tcore = np.ascontiguousarray(np.concatenate(tg_parts, axis=1))
        in_maps.append({"x": xf, "gidx": idx16, "tgt": tcore})

    caps_t = tuple(int(v) for v in caps)
    return caps_t, in_maps, core_of[blk], pos_of[blk] * P + slot


def _run(x, edge_index, trace=False):
    from concourse.bass_utils import run_bass_kernel_spmd

    caps_t, in_maps, node_core, node_row = _host_prep(x, edge_index)
    key = ("prog", caps_t, MM_DT)
    if key not in _CACHE:
        nc_ = _build_program(caps_t, MM_DT)
        nc_.finalize()
        _CACHE[key] = nc_
    nc = _CACHE[key]
    res = run_bass_kernel_spmd(
        nc, in_maps, core_ids=list(range(NCORES)), trace=trace)

    outs = [np.asarray(r["out"]) for r in res.results]
    out_full = np.empty((N_NODES, N_FEAT), np.float32)
    for c in range(NCORES):
        m = node_core == c
        out_full[m] = outs[c][node_row[m]]
    return out_full, res


def kernel(**inputs):
    out, _ = _run(inputs["x"], inputs["edge_index"], trace=False)
    return out


# revision 22
# speedup vs baseline: 1.0400x; 1.0400x over previous
"""GNN message-passing (std aggregator) on 8 TRN2 NeuronCores.

Math per target node: count, S1 = sum x[src], S2 = sum x[src]^2;
mean = S1/max(count,eps); var = S2/count - mean^2;
std = sqrt(max(var,0)), zeroed where count <= 1.

Strategy: shard TARGET nodes across cores (no collectives). Host packs nodes
into 128-bin blocks with a greedy 4-dim balancer (per-quarter loads <= ~512),
sorts blocks by load and deals them serpentine to cores so every core has the
same per-position load profile. Each block position gets its own compile-time
capacity (128-multiple), so gather padding is ~2-3% instead of 25%. Per group
of GB blocks and src-quarter q there is ONE dma_gather (int16 idx < 25000);
gathers round-robin 4 SWDGE queues so 4 GpSimd Q7 pairs emit descriptors
concurrently (~3.2x). Per group: ACT builds [x | x^2 | 1] bf16 rhs, DVE builds
one-hot tiles (label-vs-iota is_equal), PE accumulates [128 x 129] = [S1 | S2
| count] per block in PSUM, then a batched finishing pass computes std and one
strided DMA per group writes out.
"""

import numpy as np

N_NODES = 100000
N_FEAT = 64
N_EDGES = 1600000
P = 128
NCORES = 8
NB = 98                 # blocks per core
NBLK = NCORES * NB      # 784
GB = 7                  # blocks per group; 98 = 14*7
NG = NB // GB
NQUART = 4
NQ = N_NODES // NQUART  # rows per src quarter (25000 < 32768 for int16 idx)
EPS = 1e-8
MM_DT = "bfloat16"      # matmul operand dtype

_CACHE = {}


def _build_program(caps, mm_dt):
    """caps: tuple of NB ints, capacity (multiple of 128) per block position."""
    import concourse.bacc as bacc
    import concourse.mybir as mybir
    import concourse.tile as tile

    F32 = mybir.dt.float32
    I16 = mybir.dt.int16
    MDT = getattr(mybir.dt, mm_dt)
    AO = mybir.AluOpType
    AF = mybir.ActivationFunctionType

    f = N_FEAT
    W = 2 * f + 1
    tiles = [c // P for c in caps]               # tile-columns per (pos, q)
    # per-group geometry
    gtiles = [sum(tiles[g * GB:(g + 1) * GB]) for g in range(NG)]  # per q
    gcols_g = [4 * t for t in gtiles]            # tile-cols per group
    maxgt = max(gtiles)
    maxgc = max(gcols_g)
    C = sum(gcols_g)                             # total columns per core
    i16_gq = [t * P // 16 for t in gtiles]       # idx16 cols per (g, q) gather
    IC = 4 * sum(i16_gq)                         # idx16 cols per core

    nc = bacc.Bacc(num_swdge_queues=4)
    xd = nc.declare_dram_parameter("x", [N_NODES, f], F32, isOutput=False)
    gidxd = nc.declare_dram_parameter("gidx", [P, IC], I16, isOutput=False)
    tgtd = nc.declare_dram_parameter("tgt", [P, C], MDT, isOutput=False)
    outd = nc.declare_dram_parameter("out", [NB * P, f], F32, isOutput=True)

    with tile.TileContext(nc) as tc:
        with (
            tc.tile_pool(name="const", bufs=1) as constp,
            tc.tile_pool(name="msg", bufs=2) as msgp,
            tc.tile_pool(name="oh", bufs=2) as ohp,
            tc.tile_pool(name="fin", bufs=2) as finp,
            tc.tile_pool(name="ov", bufs=2) as ovp,
            tc.tile_pool(name="ps", bufs=8, space="PSUM") as psump,
        ):
            iotat = constp.tile([P, maxgt * P], MDT)
            nc.gpsimd.iota(iotat[:], pattern=[[0, maxgt], [1, P]], base=0,
                           channel_multiplier=0,
                           allow_small_or_imprecise_dtypes=True)

            idxall = constp.tile([P, IC], I16)
            c0 = 4 * i16_gq[0]
            nc.sync.dma_start(out=idxall[:, 0:c0], in_=gidxd[:, 0:c0])
            nc.sync.dma_start(out=idxall[:, c0:], in_=gidxd[:, c0:])
            tgall = constp.tile([P, C], MDT)
            nc.sync.dma_start(out=tgall[:], in_=tgtd[:, :])

            out3 = outd[:].rearrange("(b p) f -> p b f", p=P)

            def _drain(pst):
                fin = finp.tile([P, GB * W], F32, tag="fin")
                for j, pt in enumerate(pst):
                    nc.scalar.activation(out=fin[:, j * W:(j + 1) * W],
                                         in_=pt[:], func=AF.Copy)
                f3 = fin[:].rearrange("p (b w) -> p b w", w=W)
                cnt = finp.tile([P, GB], F32, tag="cnt")
                nc.scalar.activation(
                    out=cnt[:].rearrange("p (b u) -> p b u", u=1),
                    in_=f3[:, :, 2 * f:2 * f + 1],
                    func=AF.Copy, bias=float(EPS))
                return fin, cnt

            def _math(fin, cnt, ooff):
                f3 = fin[:].rearrange("p (b w) -> p b w", w=W)
                rec = finp.tile([P, GB], F32, tag="rec")
                nc.vector.reciprocal(out=rec[:], in_=cnt[:])
                r3 = rec[:].rearrange("p (b u) -> p b u", u=1)
                mom = finp.tile([P, GB * 2 * f], F32, tag="mom")
                m3 = mom[:].rearrange("p (b w) -> p b w", w=2 * f)
                nc.vector.tensor_tensor(
                    out=m3[:, :, :], in0=f3[:, :, 0:2 * f],
                    in1=r3.to_broadcast([P, GB, 2 * f]), op=AO.mult)
                var = finp.tile([P, GB * f], F32, tag="var")
                v3 = var[:].rearrange("p (b w) -> p b w", w=f)
                nc.vector.tensor_tensor(
                    out=v3[:, :, :], in0=m3[:, :, 0:f], in1=m3[:, :, 0:f],
                    op=AO.mult)
                nc.vector.tensor_tensor(
                    out=v3[:, :, :], in0=m3[:, :, f:2 * f], in1=v3[:, :, :],
                    op=AO.subtract)
                std = ovp.tile([P, GB * f], F32, tag="std")
                nc.scalar.activation(out=std[:], in_=var[:], func=AF.Relu)
                nc.scalar.sqrt(out=std[:], in_=std[:])
                mask = finp.tile([P, GB], F32, tag="mask")
                nc.vector.tensor_scalar(
                    out=mask[:], in0=cnt[:],
                    scalar1=1.5, scalar2=None, op0=AO.is_gt)
                s3o = std[:].rearrange("p (b w) -> p b w", w=f)
                nc.vector.tensor_tensor(
                    out=s3o[:, :, :], in0=s3o[:, :, :],
                    in1=mask[:].rearrange("p (b u) -> p b u", u=1)
                        .to_broadcast([P, GB, f]),
                    op=AO.mult)
                nc.sync.dma_start(
                    out=out3[:, ooff:ooff + GB, :], in_=s3o[:, :, :])

            pending = []
            ioff = 0   # idx16 column offset
            coff = 0   # tgt column offset
            ooff = 0   # out block offset
            for g in range(NG):
                gt = gtiles[g]
                gc = gcols_g[g]
                i16g = i16_gq[g]

                if len(pending) == 2:
                    _math(*pending.pop(0))

                gbuf = msgp.tile([P, maxgc * f], F32, tag="g")
                g3 = gbuf[:].rearrange("p (c e) -> p c e", e=f)
                for q in range(NQUART):
                    nc.gpsimd.dma_gather(
                        out_ap=g3[:, q * gt:(q + 1) * gt, :],
                        in_ap=xd[q * NQ:(q + 1) * NQ, :],
                        idxs_ap=idxall[:, ioff + q * i16g:ioff + (q + 1) * i16g],
                        num_idxs=gt * P,
                        num_idxs_reg=gt * P,
                        elem_size=f,
                        single_packet=False,
                        queue_num=q,
                    )

                sqx = msgp.tile([P, maxgc * W], MDT, tag="sqx")
                s3 = sqx[:].rearrange("p (c w) -> p c w", w=W)
                pst = [psump.tile([P, W], F32, tag="ps",
                                  name=f"ps_{g}_{j}") for j in range(GB)]
                pss = [pt[:] for pt in pst]
                for q in range(NQUART):
                    sl = slice(q * gt, (q + 1) * gt)
                    nc.scalar.activation(out=s3[:, sl, 0:f], in_=g3[:, sl, :],
                                         func=AF.Copy)
                    nc.scalar.square(out=s3[:, sl, f:2 * f], in_=g3[:, sl, :])
                    nc.scalar.activation(out=s3[:, sl, 2 * f:W],
                                         in_=g3[:, sl, 0:1],
                                         func=AF.Copy, bias=1.0, scale=0.0)
                    oh = ohp.tile([P, maxgt * P], MDT)
                    nc.vector.tensor_tensor(
                        out=oh[:, 0:gt * P].rearrange("p (c e) -> p c e", e=P),
                        in0=tgall[:, coff + q * gt:coff + (q + 1) * gt]
                            .rearrange("p (c u) -> p c u", u=1)
                            .to_broadcast([P, gt, P]),
                        in1=iotat[:, 0:gt * P]
                            .rearrange("p (c e) -> p c e", e=P),
                        op=AO.is_equal,
                    )
                    toff = 0
                    for bl in range(GB):
                        nt = tiles[g * GB + bl]
                        for t in range(nt):
                            cl = q * gt + toff + t
                            nc.tensor.matmul(
                                out=pss[bl],
                                lhsT=oh[:, (toff + t) * P:(toff + t + 1) * P],
                                rhs=sqx[:, cl * W:(cl + 1) * W],
                                start=(q == 0 and t == 0),
                                stop=(q == NQUART - 1 and t == nt - 1),
                            )
                        toff += nt

                # finishing deferred one group so its DVE/ACT ops never
                # stall the next group's one-hot builds
                pending.append(_drain(pst) + (ooff,))

                ioff += 4 * i16g
                coff += gc
                ooff += GB
            while pending:
                _math(*pending.pop(0))
    return nc


def _balance(deg4):
    """Greedy 4-dim balanced assignment of nodes to NBLK blocks (<=128 each)."""
    tot = deg4.sum(1)
    order = np.argsort(-tot, kind="stable")
    loads = np.zeros((NBLK, NQUART), np.int32)
    cnt = np.zeros(NBLK, np.int32)
    blk = np.empty(N_NODES, np.int64)
    slot = np.empty(N_NODES, np.int64)
    full = np.zeros(NBLK, bool)
    CAP = 512
    for n in order:
        cand = loads + deg4[n]
        mx = cand.max(axis=1)
        sc = np.where((cand > CAP).any(axis=1) | full, np.inf, mx)
        b = int(np.argmin(sc))
        if np.isinf(sc[b]):
            sc2 = np.where(full, np.inf, mx)
            b = int(np.argmin(sc2))
        blk[n] = b
        slot[n] = cnt[b]
        loads[b] += deg4[n]
        cnt[b] += 1
        if cnt[b] >= P:
            full[b] = True
    return blk, slot, loads


def _host_prep(x, edge_index):
    src = np.asarray(edge_index[0], dtype=np.int64)
    tgt = np.asarray(edge_index[1], dtype=np.int64)
    n_edges = src.shape[0]

    eq = src // NQ
    deg4 = np.bincount(tgt * NQUART + eq,
                       minlength=N_NODES * NQUART).reshape(N_NODES, NQUART)
    blk, slot, loads = _balance(deg4.astype(np.int32))

    # sort blocks by max quarter load desc, serpentine-deal to cores so each
    # core's position profile matches; capacity per position = max over cores
    bmax = loads.max(axis=1)
    border = np.argsort(-bmax, kind="stable")    # global block rank
    rank_of = np.empty(NBLK, np.int64)
    rank_of[border] = np.arange(NBLK)
    rounds = rank_of // NCORES
    posn = rank_of % NCORES
    core_of = np.where(rounds % 2 == 0, posn, NCORES - 1 - posn)
    pos_of = rounds                              # block position within core

    # per-position capacity (multiple of 128), same for all cores
    segmax = np.zeros(NB, np.int64)
    np.maximum.at(segmax, pos_of, bmax)
    caps = (np.ceil(np.maximum(segmax, 1) / P).astype(np.int64) * P)

    # per-edge placement
    eb = blk[tgt]
    ecore = core_of[eb]
    epos = pos_of[eb]
    es = slot[tgt]
    # segment id in stream order: (core, group, q, block-in-group)
    egrp = epos // GB
    ebl = epos % GB
    seg = ((ecore * NG + egrp) * NQUART + eq) * GB + ebl
    nseg = NCORES * NG * NQUART * GB
    # capacity per segment id
    segcap = np.empty(nseg, np.int64)
    sid = np.arange(nseg)
    segcap[:] = caps[(sid // (NQUART * GB)) % NG * GB + sid % GB]
    segstart = np.zeros(nseg, np.int64)
    np.cumsum(segcap[:-1], out=segstart[1:])
    total = int(segcap.sum())

    segsums = np.bincount(seg, minlength=nseg)
    assert (segsums <= segcap).all()

    order_e = np.argsort(seg, kind="stable")
    segs = seg[order_e]
    starts = np.zeros(nseg, np.int64)
    np.cumsum(segsums[:-1], out=starts[1:])
    within = np.arange(n_edges) - starts[segs]
    flat = segstart[segs] + within

    gidx_all = np.zeros(total, np.int16)
    tgt_f = np.full(total, -1.0, np.float32)
    gidx_all[flat] = (src[order_e] % NQ).astype(np.int16)
    tgt_f[flat] = es[order_e].astype(np.float32)
    # bf16 encoding: upper 16 bits of float32 (values 0..127/-1 are exact)
    tgt_all = (tgt_f.view(np.uint32) >> 16).astype(np.uint16).view(np.int16)

    xf = np.ascontiguousarray(np.asarray(x, dtype=np.float32))
    per_core = total // NCORES
    # per (core, g, q): gather stream = GB consecutive segments
    # idx16 wrap: [cap16, 16] -> [16, cap16], replicate x8
    tiles = caps // P
    gtiles = [int(tiles[g * GB:(g + 1) * GB].sum()) for g in range(NG)]

    in_maps = []
    for c in range(NCORES):
        base = c * per_core
        gi_parts = []
        tg_parts = []
        off = base
        for g in range(NG):
            for q in range(NQUART):
                L = gtiles[g] * P
                sidx = gidx_all[off:off + L]
                stgt = tgt_all[off:off + L]
                gi_parts.append(sidx.reshape(L // 16, 16).T)
                tg_parts.append(stgt.reshape(gtiles[g], P).T)
                off += L
        gi16 = np.concatenate(gi_parts, axis=1)          # [16, IC]
        idx16 = np.ascontiguousarray(np.tile(gi16, (8, 1)))
        tcore = np.ascontiguousarray(np.concatenate(tg_parts, axis=1))
        in_maps.append({"x": xf, "gidx": idx16, "tgt": tcore})

    caps_t = tuple(int(v) for v in caps)
    return caps_t, in_maps, core_of[blk], pos_of[blk] * P + slot


def _run(x, edge_index, trace=False):
    from concourse.bass_utils import run_bass_kernel_spmd

    caps_t, in_maps, node_core, node_row = _host_prep(x, edge_index)
    key = ("prog", caps_t, MM_DT)
    if key not in _CACHE:
        nc_ = _build_program(caps_t, MM_DT)
        nc_.finalize()
        _CACHE[key] = nc_
    nc = _CACHE[key]
    res = run_bass_kernel_spmd(
        nc, in_maps, core_ids=list(range(NCORES)), trace=trace)

    outs = [np.asarray(r["out"]) for r in res.results]
    out_full = np.empty((N_NODES, N_FEAT), np.float32)
    for c in range(NCORES):
        m = node_core == c
        out_full[m] = outs[c][node_row[m]]
    return out_full, res


def kernel(**inputs):
    out, _ = _run(inputs["x"], inputs["edge_index"], trace=False)
    return out


# revision 24
# speedup vs baseline: 1.0834x; 1.0417x over previous
"""GNN message-passing (std aggregator) on 8 TRN2 NeuronCores.

Math per target node: count, S1 = sum x[src], S2 = sum x[src]^2;
mean = S1/max(count,eps); var = S2/count - mean^2;
std = sqrt(max(var,0)), zeroed where count <= 1.

Strategy: shard TARGET nodes across cores (no collectives). Host packs nodes
into 128-bin blocks with a greedy 4-dim balancer (per-quarter loads <= ~512),
sorts blocks by load and deals them serpentine to cores so every core has the
same per-position load profile. Each block position gets its own compile-time
capacity (128-multiple), so gather padding is ~2-3% instead of 25%. Per group
of GB blocks and src-quarter q there is ONE dma_gather (int16 idx < 25000);
gathers round-robin 4 SWDGE queues so 4 GpSimd Q7 pairs emit descriptors
concurrently (~3.2x). Per group: ACT builds [x | x^2 | 1] bf16 rhs, DVE builds
one-hot tiles (label-vs-iota is_equal), PE accumulates [128 x 129] = [S1 | S2
| count] per block in PSUM, then a batched finishing pass computes std and one
strided DMA per group writes out.
"""

import numpy as np

N_NODES = 100000
N_FEAT = 64
N_EDGES = 1600000
P = 128
NCORES = 8
NB = 98                 # blocks per core
NBLK = NCORES * NB      # 784
GRPS = [1, 2, 4] + [7] * 12 + [4, 2, 1]   # per-group block counts (sum 98)
GBMAX = max(GRPS)
NGRP = len(GRPS)
NQUART = 4
NQ = N_NODES // NQUART  # rows per src quarter (25000 < 32768 for int16 idx)
EPS = 1e-8
MM_DT = "bfloat16"      # matmul operand dtype

_CACHE = {}


def _build_program(caps, mm_dt):
    """caps: tuple of NB ints, capacity (multiple of 128) per block position."""
    import concourse.bacc as bacc
    import concourse.mybir as mybir
    import concourse.tile as tile

    F32 = mybir.dt.float32
    I16 = mybir.dt.int16
    MDT = getattr(mybir.dt, mm_dt)
    AO = mybir.AluOpType
    AF = mybir.ActivationFunctionType

    f = N_FEAT
    W = 2 * f + 1
    tiles = [c // P for c in caps]               # tile-columns per (pos, q)
    # per-group geometry (variable group sizes)
    gstart = np.concatenate([[0], np.cumsum(GRPS)])
    gtiles = [sum(tiles[gstart[g]:gstart[g + 1]]) for g in range(len(GRPS))]
    gcols_g = [4 * t for t in gtiles]            # tile-cols per group
    maxgt = max(gtiles)
    maxgc = max(gcols_g)
    C = sum(gcols_g)                             # total columns per core
    i16_gq = [t * P // 16 for t in gtiles]       # idx16 cols per (g, q) gather
    IC = 4 * sum(i16_gq)                         # idx16 cols per core

    nc = bacc.Bacc(num_swdge_queues=4)
    xd = nc.declare_dram_parameter("x", [N_NODES, f], F32, isOutput=False)
    gidxd = nc.declare_dram_parameter("gidx", [P, IC], I16, isOutput=False)
    tgtd = nc.declare_dram_parameter("tgt", [P, C], MDT, isOutput=False)
    outd = nc.declare_dram_parameter("out", [NB * P, f], F32, isOutput=True)

    with tile.TileContext(nc) as tc:
        with (
            tc.tile_pool(name="const", bufs=1) as constp,
            tc.tile_pool(name="msg", bufs=2) as msgp,
            tc.tile_pool(name="oh", bufs=2) as ohp,
            tc.tile_pool(name="fin", bufs=2) as finp,
            tc.tile_pool(name="ov", bufs=2) as ovp,
            tc.tile_pool(name="ps", bufs=8, space="PSUM") as psump,
        ):
            iotat = constp.tile([P, maxgt * P], MDT)
            nc.gpsimd.iota(iotat[:], pattern=[[0, maxgt], [1, P]], base=0,
                           channel_multiplier=0,
                           allow_small_or_imprecise_dtypes=True)

            idxall = constp.tile([P, IC], I16)
            c0 = 4 * i16_gq[0]
            nc.sync.dma_start(out=idxall[:, 0:c0], in_=gidxd[:, 0:c0])
            nc.sync.dma_start(out=idxall[:, c0:], in_=gidxd[:, c0:])
            tgall = constp.tile([P, C], MDT)
            nc.sync.dma_start(out=tgall[:], in_=tgtd[:, :])

            out3 = outd[:].rearrange("(b p) f -> p b f", p=P)

            def _drain(pst):
                gb = len(pst)
                fin = finp.tile([P, GBMAX * W], F32, tag="fin")
                for j, pt in enumerate(pst):
                    nc.scalar.activation(out=fin[:, j * W:(j + 1) * W],
                                         in_=pt[:], func=AF.Copy)
                f3 = fin[:, 0:gb * W].rearrange("p (b w) -> p b w", w=W)
                cnt = finp.tile([P, GBMAX], F32, tag="cnt")
                nc.scalar.activation(
                    out=cnt[:, 0:gb].rearrange("p (b u) -> p b u", u=1),
                    in_=f3[:, :, 2 * f:2 * f + 1],
                    func=AF.Copy, bias=float(EPS))
                return fin, cnt, gb

            def _math(fin, cnt, gb, ooff):
                f3 = fin[:, 0:gb * W].rearrange("p (b w) -> p b w", w=W)
                rec = finp.tile([P, GBMAX], F32, tag="rec")
                nc.vector.reciprocal(out=rec[:, 0:gb], in_=cnt[:, 0:gb])
                r3 = rec[:, 0:gb].rearrange("p (b u) -> p b u", u=1)
                mom = finp.tile([P, GBMAX * 2 * f], F32, tag="mom")
                m3 = mom[:, 0:gb * 2 * f].rearrange("p (b w) -> p b w",
                                                    w=2 * f)
                nc.vector.tensor_tensor(
                    out=m3[:, :, :], in0=f3[:, :, 0:2 * f],
                    in1=r3.to_broadcast([P, gb, 2 * f]), op=AO.mult)
                var = finp.tile([P, GBMAX * f], F32, tag="var")
                v3 = var[:, 0:gb * f].rearrange("p (b w) -> p b w", w=f)
                nc.vector.tensor_tensor(
                    out=v3[:, :, :], in0=m3[:, :, 0:f], in1=m3[:, :, 0:f],
                    op=AO.mult)
                nc.vector.tensor_tensor(
                    out=v3[:, :, :], in0=m3[:, :, f:2 * f], in1=v3[:, :, :],
                    op=AO.subtract)
                std = ovp.tile([P, GBMAX * f], F32, tag="std")
                nc.scalar.activation(out=std[:, 0:gb * f],
                                     in_=var[:, 0:gb * f], func=AF.Relu)
                nc.scalar.sqrt(out=std[:, 0:gb * f], in_=std[:, 0:gb * f])
                mask = finp.tile([P, GBMAX], F32, tag="mask")
                nc.vector.tensor_scalar(
                    out=mask[:, 0:gb], in0=cnt[:, 0:gb],
                    scalar1=1.5, scalar2=None, op0=AO.is_gt)
                s3o = std[:, 0:gb * f].rearrange("p (b w) -> p b w", w=f)
                nc.vector.tensor_tensor(
                    out=s3o[:, :, :], in0=s3o[:, :, :],
                    in1=mask[:, 0:gb].rearrange("p (b u) -> p b u", u=1)
                        .to_broadcast([P, gb, f]),
                    op=AO.mult)
                nc.sync.dma_start(
                    out=out3[:, ooff:ooff + gb, :], in_=s3o[:, :, :])

            pending = []
            ioff = 0   # idx16 column offset
            coff = 0   # tgt column offset
            ooff = 0   # out block offset
            for g in range(len(GRPS)):
                gb = GRPS[g]
                gt = gtiles[g]
                gc = gcols_g[g]
                i16g = i16_gq[g]

                if len(pending) == 2:
                    _math(*pending.pop(0))

                gbuf = msgp.tile([P, maxgc * f], F32, tag="g")
                g3 = gbuf[:].rearrange("p (c e) -> p c e", e=f)
                for q in range(NQUART):
                    nc.gpsimd.dma_gather(
                        out_ap=g3[:, q * gt:(q + 1) * gt, :],
                        in_ap=xd[q * NQ:(q + 1) * NQ, :],
                        idxs_ap=idxall[:, ioff + q * i16g:ioff + (q + 1) * i16g],
                        num_idxs=gt * P,
                        num_idxs_reg=gt * P,
                        elem_size=f,
                        single_packet=False,
                        queue_num=q,
                    )

                sqx = msgp.tile([P, maxgc * W], MDT, tag="sqx")
                s3 = sqx[:].rearrange("p (c w) -> p c w", w=W)
                pst = [psump.tile([P, W], F32, tag="ps",
                                  name=f"ps_{g}_{j}") for j in range(gb)]
                pss = [pt[:] for pt in pst]
                for q in range(NQUART):
                    sl = slice(q * gt, (q + 1) * gt)
                    nc.scalar.activation(out=s3[:, sl, 0:f], in_=g3[:, sl, :],
                                         func=AF.Copy)
                    nc.scalar.square(out=s3[:, sl, f:2 * f], in_=g3[:, sl, :])
                    nc.scalar.activation(out=s3[:, sl, 2 * f:W],
                                         in_=g3[:, sl, 0:1],
                                         func=AF.Copy, bias=1.0, scale=0.0)
                    oh = ohp.tile([P, maxgt * P], MDT)
                    nc.vector.tensor_tensor(
                        out=oh[:, 0:gt * P].rearrange("p (c e) -> p c e", e=P),
                        in0=tgall[:, coff + q * gt:coff + (q + 1) * gt]
                            .rearrange("p (c u) -> p c u", u=1)
                            .to_broadcast([P, gt, P]),
                        in1=iotat[:, 0:gt * P]
                            .rearrange("p (c e) -> p c e", e=P),
                        op=AO.is_equal,
                    )
                    toff = 0
                    for bl in range(gb):
                        nt = tiles[gstart[g] + bl]
                        for t in range(nt):
                            cl = q * gt + toff + t
                            nc.tensor.matmul(
                                out=pss[bl],
                                lhsT=oh[:, (toff + t) * P:(toff + t + 1) * P],
                                rhs=sqx[:, cl * W:(cl + 1) * W],
                                start=(q == 0 and t == 0),
                                stop=(q == NQUART - 1 and t == nt - 1),
                            )
                        toff += nt

                # finishing deferred one group so its DVE/ACT ops never
                # stall the next group's one-hot builds
                pending.append(_drain(pst) + (ooff,))

                ioff += 4 * i16g
                coff += gc
                ooff += gb
            while pending:
                _math(*pending.pop(0))
    return nc


def _balance(deg4):
    """Greedy 4-dim balanced assignment of nodes to NBLK blocks (<=128 each)."""
    tot = deg4.sum(1)
    order = np.argsort(-tot, kind="stable")
    loads = np.zeros((NBLK, NQUART), np.int32)
    cnt = np.zeros(NBLK, np.int32)
    blk = np.empty(N_NODES, np.int64)
    slot = np.empty(N_NODES, np.int64)
    full = np.zeros(NBLK, bool)
    CAP = 512
    for n in order:
        cand = loads + deg4[n]
        mx = cand.max(axis=1)
        sc = np.where((cand > CAP).any(axis=1) | full, np.inf, mx)
        b = int(np.argmin(sc))
        if np.isinf(sc[b]):
            sc2 = np.where(full, np.inf, mx)
            b = int(np.argmin(sc2))
        blk[n] = b
        slot[n] = cnt[b]
        loads[b] += deg4[n]
        cnt[b] += 1
        if cnt[b] >= P:
            full[b] = True
    return blk, slot, loads


def _host_prep(x, edge_index):
    src = np.asarray(edge_index[0], dtype=np.int64)
    tgt = np.asarray(edge_index[1], dtype=np.int64)
    n_edges = src.shape[0]

    eq = src // NQ
    deg4 = np.bincount(tgt * NQUART + eq,
                       minlength=N_NODES * NQUART).reshape(N_NODES, NQUART)
    blk, slot, loads = _balance(deg4.astype(np.int32))

    # sort blocks by max quarter load desc, serpentine-deal to cores so each
    # core's position profile matches; capacity per position = max over cores
    bmax = loads.max(axis=1)
    border = np.argsort(-bmax, kind="stable")    # global block rank
    rank_of = np.empty(NBLK, np.int64)
    rank_of[border] = np.arange(NBLK)
    rounds = rank_of // NCORES
    posn = rank_of % NCORES
    core_of = np.where(rounds % 2 == 0, posn, NCORES - 1 - posn)
    pos_of = rounds                              # block position within core

    # per-position capacity (multiple of 128), same for all cores
    segmax = np.zeros(NB, np.int64)
    np.maximum.at(segmax, pos_of, bmax)
    caps = (np.ceil(np.maximum(segmax, 1) / P).astype(np.int64) * P)

    # per-edge placement
    eb = blk[tgt]
    ecore = core_of[eb]
    epos = pos_of[eb]
    es = slot[tgt]
    # segment id in stream order: (core, group, q, block-in-group)
    gstart = np.concatenate([[0], np.cumsum(GRPS)])
    g_of_pos = np.empty(NB, np.int64)
    bl_of_pos = np.empty(NB, np.int64)
    for g in range(NGRP):
        g_of_pos[gstart[g]:gstart[g + 1]] = g
        bl_of_pos[gstart[g]:gstart[g + 1]] = np.arange(GRPS[g])
    egrp = g_of_pos[epos]
    ebl = bl_of_pos[epos]
    seg = ((ecore * NGRP + egrp) * NQUART + eq) * GBMAX + ebl
    nseg = NCORES * NGRP * NQUART * GBMAX
    # capacity per segment id (0 for unused block-in-group slots)
    sid = np.arange(nseg)
    sg = (sid // (NQUART * GBMAX)) % NGRP
    sbl = sid % GBMAX
    valid = sbl < np.asarray(GRPS)[sg]
    spos = np.where(valid, gstart[sg] + np.minimum(sbl, np.asarray(GRPS)[sg] - 1), 0)
    segcap = np.where(valid, caps[spos], 0)
    segstart = np.zeros(nseg, np.int64)
    np.cumsum(segcap[:-1], out=segstart[1:])
    total = int(segcap.sum())

    segsums = np.bincount(seg, minlength=nseg)
    assert (segsums <= segcap).all()

    order_e = np.argsort(seg, kind="stable")
    segs = seg[order_e]
    starts = np.zeros(nseg, np.int64)
    np.cumsum(segsums[:-1], out=starts[1:])
    within = np.arange(n_edges) - starts[segs]
    flat = segstart[segs] + within

    gidx_all = np.zeros(total, np.int16)
    tgt_f = np.full(total, -1.0, np.float32)
    gidx_all[flat] = (src[order_e] % NQ).astype(np.int16)
    tgt_f[flat] = es[order_e].astype(np.float32)
    # bf16 encoding: upper 16 bits of float32 (values 0..127/-1 are exact)
    tgt_all = (tgt_f.view(np.uint32) >> 16).astype(np.uint16).view(np.int16)

    xf = np.ascontiguousarray(np.asarray(x, dtype=np.float32))
    per_core = total // NCORES
    # per (core, g, q): gather stream = the group's consecutive segments
    # idx16 wrap: [cap16, 16] -> [16, cap16], replicate x8
    tiles = caps // P
    gtiles = [int(tiles[gstart[g]:gstart[g + 1]].sum()) for g in range(NGRP)]

    in_maps = []
    for c in range(NCORES):
        base = c * per_core
        gi_parts = []
        tg_parts = []
        off = base
        for g in range(NGRP):
            for q in range(NQUART):
                L = gtiles[g] * P
                sidx = gidx_all[off:off + L]
                stgt = tgt_all[off:off + L]
                gi_parts.append(sidx.reshape(L // 16, 16).T)
                tg_parts.append(stgt.reshape(gtiles[g], P).T)
                off += L
        gi16 = np.concatenate(gi_parts, axis=1)          # [16, IC]
        idx16 = np.ascontiguousarray(np.tile(gi16, (8, 1)))
        tcore = np.ascontiguousarray(np.concatenate(tg_parts, axis=1))
        in_maps.append({"x": xf, "gidx": idx16, "tgt": tcore})

    caps_t = tuple(int(v) for v in caps)
    return caps_t, in_maps, core_of[blk], pos_of[blk] * P + slot


def _run(x, edge_index, trace=False):
    from concourse.bass_utils import run_bass_kernel_spmd

    caps_t, in_maps, node_core, node_row = _host_prep(x, edge_index)
    key = ("prog", caps_t, MM_DT)
    if key not in _CACHE:
        nc_ = _build_program(caps_t, MM_DT)
        nc_.finalize()
        _CACHE[key] = nc_
    nc = _CACHE[key]
    res = run_bass_kernel_spmd(
        nc, in_maps, core_ids=list(range(NCORES)), trace=trace)

    outs = [np.asarray(r["out"]) for r in res.results]
    out_full = np.empty((N_NODES, N_FEAT), np.float32)
    for c in range(NCORES):
        m = node_core == c
        out_full[m] = outs[c][node_row[m]]
    return out_full, res


def kernel(**inputs):
    out, _ = _run(inputs["x"], inputs["edge_index"], trace=False)
    return out
